# revision 1
# baseline (speedup 1.0000x reference)
"""Trainium2 Bass kernel for nn_ContextPythiaModel (ContextBlock + 6-layer
Pythia + vocab head).  See build notes in repo test.py.

Two SPMD launches on 8 cores:
  Launch 1 - sequential context recurrence, split speculatively across
    cores (perturbations decay below fp32 noise in ~30 steps; each core
    runs a 32-step warm-up from a zero context before its 122-token chunk;
    core 0 starts exactly from prev_context).
  Launch 2 - core = (batch b, vocab half): all 6 transformer layers for
    batch b (duplicated x2: attention needs the full sequence), then the
    [S, V/2] logits slice.

LayerNorm scales/means are folded into adjacent weights on the host;
rsqrt runs on the vector engine (bit trick + Newton) so the scalar engine
never switches activation-table sets inside hot loops.
"""
import sys, os
os.environ.setdefault('TRN_TOPOLOGY', 'trn2.8x1')
os.environ['BASS_NEVER_TRACE'] = '1'  # axon client lacks the NTFF hook
if '/opt/trn_rl_repo' not in sys.path:
    sys.path.insert(0, '/opt/trn_rl_repo')

import numpy as np
import ml_dtypes
import concourse.bass as bass
from concourse import bacc
import concourse.mybir as mybir
from concourse.tile import TileContext
from concourse.masks import make_identity
from concourse.bass_utils import run_bass_kernel_spmd

P = 128
F32 = mybir.dt.float32
BF16 = mybir.dt.bfloat16
I32 = mybir.dt.int32
AF = mybir.ActivationFunctionType
OP = mybir.AluOpType

V, E, C, L, H, I = 50304, 512, 256, 6, 8, 1024
D = C // H
ROT = 8
B, S = 4, 1024
NCORE = 8
W_WARM = 32
CHUNK = (S - W_WARM) // NCORE      # 122
T = W_WARM + CHUNK                 # 170 steps per core
TB = T * B                         # 680
TBP = ((TB + P - 1) // P) * P      # 768
NTT = TBP // P                     # 6
EPS = 1e-5
MAGIC2 = 0x5EF759DF
VH = V // 2                         # 25152
VHP = 25600                         # padded to 50*512
NVC = VHP // 512
NT = S // P


def _ts(nc, out, in0, s1, s2, op0, op1=None):
    if op1 is None:
        nc.vector.tensor_scalar(out=out, in0=in0, scalar1=s1, scalar2=None, op0=op0)
    else:
        nc.vector.tensor_scalar(out=out, in0=in0, scalar1=s1, scalar2=s2,
                                op0=op0, op1=op1)


def _tt(nc, out, in0, in1, op):
    nc.vector.tensor_tensor(out=out, in0=in0, in1=in1, op=op)


def emit_rsqrt(nc, pool, out, v, shape, tag="rsq"):
    """out = 1/sqrt(v), v>0, vector-engine only (bit trick + 2 Newton)."""
    v2 = pool.tile(shape, F32, tag=tag + "v2")
    _ts(nc, v2, v, 0.5, None, OP.mult)
    sh = pool.tile(shape, I32, tag=tag + "sh")
    _ts(nc, sh, v2.bitcast(I32), 1, None, OP.logical_shift_right)
    seed = pool.tile(shape, I32, tag=tag + "sd")
    _ts(nc, seed, sh, -1, MAGIC2, OP.mult, OP.add)
    r = seed.bitcast(F32)
    rr = pool.tile(shape, F32, tag=tag + "rr")
    w_ = pool.tile(shape, F32, tag=tag + "w")
    c_ = pool.tile(shape, F32, tag=tag + "c")
    for it in range(2):
        src = r if it == 0 else out
        _tt(nc, rr, src, src, OP.mult)
        _tt(nc, w_, rr, v2, OP.mult)
        _ts(nc, c_, w_, -1.0, 1.5, OP.mult, OP.add)
        _tt(nc, out, src, c_, OP.mult)


# ==========================================================================
# Launch 1
# ==========================================================================

def build_rec_program():
    nc = bacc.Bacc("TRN2")
    embw = nc.dram_tensor("embw", [TBP, E], F32, kind="ExternalInput")
    we3 = nc.dram_tensor("we3", [4, P, C], F32, kind="ExternalInput")
    wc3 = nc.dram_tensor("wc3", [2, P, C], F32, kind="ExternalInput")
    c0v = nc.dram_tensor("c0v", [C, 1], F32, kind="ExternalInput")
    ginit = nc.dram_tensor("ginit", [C, B], F32, kind="ExternalInput")
    hfix = nc.dram_tensor("hfix", [C, B], F32, kind="ExternalInput")
    cnw = nc.dram_tensor("cnw", [C, 1], F32, kind="ExternalInput")
    cnb = nc.dram_tensor("cnb", [C, 1], F32, kind="ExternalInput")
    xout = nc.dram_tensor("xout", [2, P, TB], F32, kind="ExternalOutput")

    with TileContext(nc) as tc:
        with tc.tile_pool(name="big", bufs=1) as big, \
             tc.tile_pool(name="sm", bufs=2) as sm, \
             tc.tile_pool(name="ps", bufs=2, space="PSUM") as ps, \
             tc.tile_pool(name="psg", bufs=2, space="PSUM") as psg:

            ident = big.tile([P, P], F32)
            make_identity(nc, ident)

            emb = big.tile([P, NTT, E], F32)
            nc.sync.dma_start(out=emb,
                              in_=embw[:, :].rearrange("(i p) e -> p i e", p=P))

            # embedding LN (mean folded into we3): scale rows by rstd
            mv = big.tile([P, NTT, 2], F32)
            for i in range(NTT):
                st = sm.tile([P, 6], F32, tag="st")
                nc.vector.bn_stats(out=st, in_=emb[:, i, :])
                nc.vector.bn_aggr(out=mv[:, i, :], in_=st)
            veps = big.tile([P, NTT], F32)
            _ts(nc, veps, mv[:, :, 1], 1.0, EPS, OP.mult, OP.add)
            r_tok = big.tile([P, NTT], F32)
            emit_rsqrt(nc, sm, r_tok, veps, [P, NTT])
            for i in range(NTT):
                _ts(nc, emb[:, i, :], emb[:, i, :], r_tok[:, i:i + 1], None, OP.mult)

            embT = big.tile([P, 4, TBP], F32)
            for i in range(NTT):
                for k in range(4):
                    pt = ps.tile([P, P], F32, tag="w")
                    nc.tensor.transpose(out=pt, in_=emb[:, i, k * P:(k + 1) * P],
                                        identity=ident)
                    nc.any.tensor_copy(out=embT[:, k, i * P:(i + 1) * P], in_=pt)

            we3_t = big.tile([P, 4, C], F32)
            nc.sync.dma_start(out=we3_t, in_=we3[:, :, :].rearrange("k p c -> p k c"))
            c0_t = big.tile([P, 2], F32)
            nc.sync.dma_start(out=c0_t, in_=c0v[:, :].rearrange("(m p) o -> p (m o)", p=P))
            pre2 = big.tile([P, 2, TBP], F32)
            cw = TBP // 2
            for m in range(2):
                for ch in range(2):
                    pp = ps.tile([P, cw], F32, tag="w")
                    for k in range(4):
                        nc.tensor.matmul(
                            out=pp, lhsT=we3_t[:, k, m * P:(m + 1) * P],
                            rhs=embT[:, k, ch * cw:(ch + 1) * cw],
                            start=(k == 0), stop=(k == 3))
                    _ts(nc, pre2[:, m, ch * cw:(ch + 1) * cw], pp,
                        1.0, c0_t[:, m:m + 1], OP.mult, OP.add)

            hf = sm.tile([P, 2, B], F32, tag="hf")
            nc.sync.dma_start(out=hf, in_=hfix[:, :].rearrange("(m p) b -> p m b", p=P))
            for m in range(2):
                _tt(nc, pre2[:, m, 0:B], pre2[:, m, 0:B], hf[:, m, :], OP.add)

            # ---------------- recurrence ----------------
            wc3_t = big.tile([P, 2, C], F32)
            nc.sync.dma_start(out=wc3_t, in_=wc3[:, :, :].rearrange("k p c -> p k c"))
            ones_t = big.tile([P, P], F32)
            nc.vector.memset(ones_t, 1.0)
            gi = sm.tile([P, 2, B], F32, tag="gi")
            nc.sync.dma_start(out=gi, in_=ginit[:, :].rearrange("(m p) b -> p m b", p=P))

            # ghist slot t = g(token s_k + t); rxh slot t = rstd of that g
            ghist = big.tile([P, 2, TB], F32)
            rxh = big.tile([P, TB], F32)
            gsq = big.tile([P, 2, 2 * B], F32)   # state [g | g^2]
            rone = big.tile([P, B], F32)
            nc.vector.memset(rone, 1.0)

            for m in range(2):
                nc.vector.tensor_copy(out=gsq[:, m, 0:B], in_=gi[:, m, :])
                _tt(nc, gsq[:, m, B:2 * B], gi[:, m, :], gi[:, m, :], OP.mult)

            for t in range(T):
                # G = Wc3 @ g_state   (g_state = g of token s_k+t-1)
                g_ps = [psg.tile([P, B], F32, tag=f"gps{m}", name=f"gps{m}_{t}")
                        for m in range(2)]
                for m in range(2):
                    for k in range(2):
                        nc.tensor.matmul(
                            out=g_ps[m], lhsT=wc3_t[:, k, m * P:(m + 1) * P],
                            rhs=gsq[:, k, 0:B], start=(k == 0), stop=(k == 1))
                if t == 0:
                    r_fix = rone
                else:
                    # stats of current state -> rstd -> rxh slot t-1
                    st_ps = psg.tile([P, 2 * B], F32, tag="stps")
                    for k in range(2):
                        nc.tensor.matmul(out=st_ps, lhsT=ones_t, rhs=gsq[:, k, :],
                                         start=(k == 0), stop=(k == 1))
                    st_sb = sm.tile([P, 2 * B], F32, tag="stsb")
                    nc.scalar.copy(out=st_sb, in_=st_ps)
                    s1 = st_sb[:, 0:B]
                    s2 = st_sb[:, B:2 * B]
                    m2 = sm.tile([P, B], F32, tag="m2")
                    _tt(nc, m2, s1, s1, OP.mult)
                    p1 = sm.tile([P, B], F32, tag="p1")
                    _ts(nc, p1, m2, -0.5 / (C * C), 0.5 * EPS, OP.mult, OP.add)
                    p2 = sm.tile([P, B], F32, tag="p2")
                    _ts(nc, p2, s2, 0.5 / C, None, OP.mult)
                    v2 = sm.tile([P, B], F32, tag="v2s")
                    _tt(nc, v2, p1, p2, OP.add)
                    sh = sm.tile([P, B], I32, tag="rsh")
                    _ts(nc, sh, v2.bitcast(I32), 1, None, OP.logical_shift_right)
                    seed = sm.tile([P, B], I32, tag="rsd")
                    _ts(nc, seed, sh, -1, MAGIC2, OP.mult, OP.add)
                    r0 = seed.bitcast(F32)
                    rr = sm.tile([P, B], F32, tag="rrr")
                    w_ = sm.tile([P, B], F32, tag="rw")
                    c_ = sm.tile([P, B], F32, tag="rc")
                    r1 = sm.tile([P, B], F32, tag="r1")
                    _tt(nc, rr, r0, r0, OP.mult)
                    _tt(nc, w_, rr, v2, OP.mult)
                    _ts(nc, c_, w_, -1.0, 1.5, OP.mult, OP.add)
                    _tt(nc, r1, r0, c_, OP.mult)
                    r_fix = rxh[:, (t - 1) * B:t * B]
                    _tt(nc, rr, r1, r1, OP.mult)
                    _tt(nc, w_, rr, v2, OP.mult)
                    _ts(nc, c_, w_, -1.0, 1.5, OP.mult, OP.add)
                    _tt(nc, r_fix, r1, c_, OP.mult)

                for m in range(2):
                    h = sm.tile([P, B], F32, tag=f"h{m}")
                    _tt(nc, h, g_ps[m], r_fix, OP.mult)
                    _tt(nc, h, h, pre2[:, m, t * B:(t + 1) * B], OP.add)
                    gdst = ghist[:, m, t * B:(t + 1) * B]
                    nc.scalar.activation(out=gdst, in_=h, func=AF.Gelu_apprx_tanh)
                    nc.vector.tensor_copy(out=gsq[:, m, 0:B], in_=gdst)
                    _tt(nc, gsq[:, m, B:2 * B], gdst, gdst, OP.mult)

            # rstd of the last g (slot T-1)
            st_l = psg.tile([P, 2 * B], F32, tag="stps")
            for k in range(2):
                nc.tensor.matmul(out=st_l, lhsT=ones_t, rhs=gsq[:, k, :],
                                 start=(k == 0), stop=(k == 1))
            st_lsb = sm.tile([P, 2 * B], F32, tag="stsb")
            nc.scalar.copy(out=st_lsb, in_=st_l)
            lm2 = sm.tile([P, B], F32, tag="m2")
            _tt(nc, lm2, st_lsb[:, 0:B], st_lsb[:, 0:B], OP.mult)
            lp1 = sm.tile([P, B], F32, tag="p1")
            _ts(nc, lp1, lm2, -1.0 / (C * C), EPS, OP.mult, OP.add)
            lp2 = sm.tile([P, B], F32, tag="p2")
            _ts(nc, lp2, st_lsb[:, B:2 * B], 1.0 / C, None, OP.mult)
            lv = sm.tile([P, B], F32, tag="v2s")
            _tt(nc, lv, lp1, lp2, OP.add)
            emit_rsqrt(nc, sm, rxh[:, (T - 1) * B:T * B], lv, [P, B], tag="rl")

            # bulk: x_t = ((g - mean)*rstd)*cn_w + cn_b  (C-major out)
            cnw_t = sm.tile([P, 2], F32, tag="cnw")
            nc.sync.dma_start(out=cnw_t, in_=cnw[:, :].rearrange("(m p) o -> p (m o)", p=P))
            cnb_t = sm.tile([P, 2], F32, tag="cnb")
            nc.sync.dma_start(out=cnb_t, in_=cnb[:, :].rearrange("(m p) o -> p (m o)", p=P))
            cw2 = TB // 2
            for m in range(2):
                for ch in range(2):
                    sl = slice(ch * cw2, (ch + 1) * cw2)
                    mm = ps.tile([P, cw2], F32, tag="w")
                    for k in range(2):
                        nc.tensor.matmul(out=mm, lhsT=ones_t,
                                         rhs=ghist[:, k, sl],
                                         start=(k == 0), stop=(k == 1))
                    msb = sm.tile([P, cw2], F32, tag="msb")
                    _ts(nc, msb, mm, 1.0 / C, None, OP.mult)
                    xo = sm.tile([P, cw2], F32, tag="xo")
                    _tt(nc, xo, ghist[:, m, sl], msb, OP.subtract)
                    _tt(nc, xo, xo, rxh[:, sl], OP.mult)
                    _ts(nc, xo, xo, cnw_t[:, m:m + 1], cnb_t[:, m:m + 1],
                        OP.mult, OP.add)
                    nc.sync.dma_start(out=xout[m, :, sl], in_=xo)
    nc.finalize()
    return nc


# ==========================================================================
# Launch 2
# ==========================================================================

def build_layers_program():
    nc = bacc.Bacc("TRN2")
    xb = nc.dram_tensor("xb", [S, C], F32, kind="ExternalInput")
    wqk = nc.dram_tensor("wqk", [L, 2, P, 2 * C], BF16, kind="ExternalInput")
    bqk = nc.dram_tensor("bqk", [L, 2 * C, 1], F32, kind="ExternalInput")
    wv = nc.dram_tensor("wv", [L, 2, P, C], BF16, kind="ExternalInput")
    wao = nc.dram_tensor("wao", [L, 2, P, C], BF16, kind="ExternalInput")
    wf1 = nc.dram_tensor("wf1", [L, 2, P, I], BF16, kind="ExternalInput")
    bf1 = nc.dram_tensor("bf1", [L, I, 1], F32, kind="ExternalInput")
    wf2 = nc.dram_tensor("wf2", [L, 8, P, C], BF16, kind="ExternalInput")
    bvo = nc.dram_tensor("bvo", [L, C, 1], F32, kind="ExternalInput")
    blay = nc.dram_tensor("blay", [L, 1, C], BF16, kind="ExternalInput")
    cosm = nc.dram_tensor("cosm", [P, S], F32, kind="ExternalInput")
    sinm = nc.dram_tensor("sinm", [P, S], F32, kind="ExternalInput")
    rsw = nc.dram_tensor("rsw", [P, P], BF16, kind="ExternalInput")
    tri01 = nc.dram_tensor("tri01", [P, P], BF16, kind="ExternalInput")
    flw = nc.dram_tensor("flw", [C, 1], F32, kind="ExternalInput")
    flb = nc.dram_tensor("flb", [C, 1], F32, kind="ExternalInput")
    outwt = nc.dram_tensor("outwt", [2, P, VHP], BF16, kind="ExternalInput")
    lg = nc.dram_tensor("lg", [S, VHP], F32, kind="ExternalOutput")

    with TileContext(nc) as tc:
        with tc.tile_pool(name="big", bufs=1) as big, \
             tc.tile_pool(name="wpool", bufs=2) as wpool, \
             tc.tile_pool(name="sm", bufs=2) as sm, \
             tc.tile_pool(name="actp", bufs=2) as actp, \
             tc.tile_pool(name="one", bufs=1) as onep, \
             tc.tile_pool(name="pex", bufs=3) as pex, \
             tc.tile_pool(name="ovw", bufs=3) as ovw, \
             tc.tile_pool(name="lgsb", bufs=4) as lgsbp, \
             tc.tile_pool(name="psw", bufs=2, space="PSUM") as psw, \
             tc.tile_pool(name="pst", bufs=2, space="PSUM") as pst, \
             tc.tile_pool(name="psa", bufs=2, space="PSUM") as psa, \
             tc.tile_pool(name="psl", bufs=2, space="PSUM") as psl:

            ident_bf = big.tile([P, P], BF16)
            make_identity(nc, ident_bf)
            ones32 = big.tile([P, D], BF16)
            nc.vector.memset(ones32, 1.0)
            ones1 = big.tile([1, P], BF16)
            nc.vector.memset(ones1, 1.0)

            x = big.tile([P, NT, C], F32)
            nc.sync.dma_start(out=x, in_=xb[:, :].rearrange("(n p) c -> p n c", p=P))
            cos_t = big.tile([P, S], F32)
            nc.sync.dma_start(out=cos_t, in_=cosm[:, :])
            sin_t = big.tile([P, S], F32)
            nc.sync.dma_start(out=sin_t, in_=sinm[:, :])
            rsw_t = big.tile([P, P], BF16)
            nc.sync.dma_start(out=rsw_t, in_=rsw[:, :])
            tri_t = big.tile([P, P], BF16)
            nc.sync.dma_start(out=tri_t, in_=tri01[:, :])

            def layernorm_T(xin, tag):
                mvb = sm.tile([P, NT, 2], F32, tag="ln_mv")
                for i in range(NT):
                    st = sm.tile([P, 6], F32, tag="ln_st")
                    nc.vector.bn_stats(out=st, in_=xin[:, i, :])
                    nc.vector.bn_aggr(out=mvb[:, i, :], in_=st)
                veps = sm.tile([P, NT], F32, tag="ln_ve")
                _ts(nc, veps, mvb[:, :, 1], 1.0, EPS, OP.mult, OP.add)
                rt = sm.tile([P, NT], F32, tag="ln_r")
                emit_rsqrt(nc, sm, rt, veps, [P, NT], tag="ln_rs")
                a_tok = sm.tile([P, NT, C], BF16, tag="ln_at")
                for i in range(NT):
                    nc.vector.tensor_scalar(
                        out=a_tok[:, i, :], in0=xin[:, i, :],
                        scalar1=mvb[:, i, 0:1], scalar2=rt[:, i:i + 1],
                        op0=OP.subtract, op1=OP.mult)
                aT = actp.tile([P, 2, S], BF16, tag=tag)
                for i in range(NT):
                    for k in range(2):
                        pt = pst.tile([P, P], BF16, tag="tp")
                        nc.tensor.transpose(out=pt, in_=a_tok[:, i, k * P:(k + 1) * P],
                                            identity=ident_bf)
                        nc.any.tensor_copy(out=aT[:, k, i * P:(i + 1) * P], in_=pt)
                return aT

            for l in range(L):
                aT = layernorm_T(x, "aT")
                mT = layernorm_T(x, "mT")

                # ---- q,k (C-major) with bias, then rope ----
                wqk_t = wpool.tile([P, 2, 2 * C], BF16, tag="wqk")
                nc.sync.dma_start(out=wqk_t, in_=wqk[l].rearrange("k p c -> p k c"))
                bqk_t = wpool.tile([P, 4], F32, tag="bqk")
                nc.sync.dma_start(out=bqk_t,
                                  in_=bqk[l].rearrange("(m p) o -> p (m o)", p=P))
                qkT = actp.tile([P, 4, S], BF16, tag="qkT")
                for m in range(4):
                    for ch in range(2):
                        sl = slice(ch * 512, (ch + 1) * 512)
                        pp = psw.tile([P, 512], F32, tag="w")
                        for k in range(2):
                            nc.tensor.matmul(
                                out=pp, lhsT=wqk_t[:, k, m * P:(m + 1) * P],
                                rhs=aT[:, k, sl], start=(k == 0), stop=(k == 1))
                        nc.scalar.activation(out=qkT[:, m, sl], in_=pp,
                                             func=AF.Identity,
                                             bias=bqk_t[:, m:m + 1], scale=1.0)
                for m in range(4):
                    for ch in range(2):
                        sl = slice(ch * 512, (ch + 1) * 512)
                        pr = psw.tile([P, 512], F32, tag="w")
                        nc.tensor.matmul(out=pr, lhsT=rsw_t, rhs=qkT[:, m, sl],
                                         start=True, stop=True)
                        t1 = sm.tile([P, 512], F32, tag="rope1")
                        _tt(nc, t1, pr, sin_t[:, sl], OP.mult)
                        t2 = sm.tile([P, 512], F32, tag="rope2")
                        _tt(nc, t2, qkT[:, m, sl], cos_t[:, sl], OP.mult)
                        _tt(nc, qkT[:, m, sl], t1, t2, OP.add)

                # ---- v (token-major) ----
                wv_t = wpool.tile([P, 2, C], BF16, tag="wv")
                nc.sync.dma_start(out=wv_t, in_=wv[l].rearrange("k p c -> p k c"))
                v_tok = actp.tile([P, NT, C], BF16, tag="v_tok")
                for i in range(NT):
                    pv = psw.tile([P, C], F32, tag="w")
                    for k in range(2):
                        nc.tensor.matmul(out=pv, lhsT=aT[:, k, i * P:(i + 1) * P],
                                         rhs=wv_t[:, k, :], start=(k == 0),
                                         stop=(k == 1))
                    nc.any.tensor_copy(out=v_tok[:, i, :], in_=pv)

                # ---- attention ----
                bvo_t = wpool.tile([P, 2], F32, tag="bvo")
                nc.sync.dma_start(out=bvo_t,
                                  in_=bvo[l].rearrange("(m p) o -> p (m o)", p=P))
                scale = float(1.0 / np.sqrt(D))
                oTf = actp.tile([P, 2, S], BF16, tag="oTf")
                for g in range(2):
                    for qc in range(2):
                        nkt = 4 * (qc + 1)
                        o_ps = psa.tile([P, 512], F32, tag="acc")
                        d_ps = psa.tile([P, 512], F32, tag="acc")
                        for kt in range(nkt):
                            o_band = max(0, kt - 4 * qc)
                            w0 = 128 * o_band
                            pe_t = pex.tile([P, 4, 512], BF16, tag="pe")
                            for j in range(4):
                                hp = slice(j * D, (j + 1) * D)
                                sps = psw.tile([P, 512], F32, tag="w")
                                nc.tensor.matmul(
                                    out=sps[:, w0:512],
                                    lhsT=qkT[hp, 2 + g, kt * P:(kt + 1) * P],
                                    rhs=qkT[hp, g, qc * 512 + w0:(qc + 1) * 512],
                                    start=True, stop=True,
                                    tile_position=(j * D, 0))
                                nc.scalar.activation(
                                    out=pe_t[:, j, w0:512], in_=sps[:, w0:512],
                                    func=AF.Exp, scale=scale)
                                if kt >= 4 * qc:
                                    _tt(nc, pe_t[:, j, w0:w0 + P],
                                        pe_t[:, j, w0:w0 + P], tri_t, OP.mult)
                            for j in range(4):
                                nc.tensor.matmul(
                                    out=o_ps[j * D:(j + 1) * D, w0:512],
                                    lhsT=v_tok[:, kt, (4 * g + j) * D:(4 * g + j + 1) * D],
                                    rhs=pe_t[:, j, w0:512],
                                    start=(kt == 0), stop=(kt == nkt - 1),
                                    tile_position=(0, j * D))
                            for j in range(4):
                                nc.tensor.matmul(
                                    out=d_ps[j * D:(j + 1) * D, w0:512],
                                    lhsT=ones32,
                                    rhs=pe_t[:, j, w0:512],
                                    start=(kt == 0), stop=(kt == nkt - 1),
                                    tile_position=(0, j * D))
                        rd = sm.tile([P, 512], F32, tag="rd")
                        nc.vector.reciprocal_approx_fast(out=rd, in_=d_ps)
                        qsl = slice(qc * 512, (qc + 1) * 512)
                        ot = sm.tile([P, 512], F32, tag="ot")
                        _tt(nc, ot, o_ps, rd, OP.mult)
                        _ts(nc, oTf[:, g, qsl], ot, 1.0, bvo_t[:, g:g + 1],
                            OP.mult, OP.add)

                # ---- mlp + residual ----
                wao_t = wpool.tile([P, 2, C], BF16, tag="wao")
                nc.sync.dma_start(out=wao_t, in_=wao[l].rearrange("k p c -> p k c"))
                wf1_t = wpool.tile([P, 2, I], BF16, tag="wf1")
                nc.sync.dma_start(out=wf1_t, in_=wf1[l].rearrange("k p c -> p k c"))
                bf1_t = wpool.tile([P, 8], F32, tag="bf1")
                nc.sync.dma_start(out=bf1_t,
                                  in_=bf1[l].rearrange("(m p) o -> p (m o)", p=P))
                wf2_t = wpool.tile([P, 8, C], BF16, tag="wf2")
                nc.sync.dma_start(out=wf2_t, in_=wf2[l].rearrange("k p c -> p k c"))
                blay_t = onep.tile([1, C], BF16, tag="blay")
                nc.sync.dma_start(out=blay_t, in_=blay[l])

                gT = actp.tile([P, 8, S], BF16, tag="gT")
                for ch in range(2):
                    sl = slice(ch * 512, (ch + 1) * 512)
                    for m in range(8):
                        pf = psw.tile([P, 512], F32, tag="w")
                        for k in range(2):
                            nc.tensor.matmul(
                                out=pf, lhsT=wf1_t[:, k, m * P:(m + 1) * P],
                                rhs=mT[:, k, sl], start=(k == 0), stop=(k == 1))
                        nc.scalar.activation(out=gT[:, m, sl], in_=pf,
                                             func=AF.Gelu_apprx_tanh,
                                             bias=bf1_t[:, m:m + 1], scale=1.0)

                for i in range(NT):
                    pr = psl.tile([P, C], F32, tag="res")
                    tsl = slice(i * P, (i + 1) * P)
                    for k in range(2):
                        nc.tensor.matmul(out=pr, lhsT=oTf[:, k, tsl],
                                         rhs=wao_t[:, k, :], start=(k == 0),
                                         stop=False)
                    for k in range(8):
                        nc.tensor.matmul(out=pr, lhsT=gT[:, k, tsl],
                                         rhs=wf2_t[:, k, :], start=False,
                                         stop=False)
                    nc.tensor.matmul(out=pr, lhsT=ones1, rhs=blay_t,
                                     start=False, stop=True)
                    _tt(nc, x[:, i, :], x[:, i, :], pr, OP.add)

            # ---- final LN + logits ----
            fT = layernorm_T(x, "fT")
            flw_t = sm.tile([P, 2], F32, tag="flw")
            nc.sync.dma_start(out=flw_t, in_=flw[:, :].rearrange("(m p) o -> p (m o)", p=P))
            flb_t = sm.tile([P, 2], F32, tag="flb")
            nc.sync.dma_start(out=flb_t, in_=flb[:, :].rearrange("(m p) o -> p (m o)", p=P))
            xhT = actp.tile([P, 2, S], BF16, tag="xhT")
            for k in range(2):
                _ts(nc, xhT[:, k, :], fT[:, k, :], flw_t[:, k:k + 1],
                    flb_t[:, k:k + 1], OP.mult, OP.add)

            for vc in range(NVC):
                vsl = slice(vc * 512, (vc + 1) * 512)
                ow = ovw.tile([P, 2, 512], BF16, tag="ow")
                nc.sync.dma_start(out=ow, in_=outwt[:, :, vsl].rearrange("k p v -> p k v"))
                for i in range(NT):
                    pl = psl.tile([P, 512], F32, tag="res")
                    for k in range(2):
                        nc.tensor.matmul(out=pl, lhsT=xhT[:, k, i * P:(i + 1) * P],
                                         rhs=ow[:, k, :], start=(k == 0),
                                         stop=(k == 1))
                    lo = lgsbp.tile([P, 512], F32, tag="lo")
                    nc.any.tensor_copy(out=lo, in_=pl)
                    nc.sync.dma_start(out=lg[i * P:(i + 1) * P, vsl], in_=lo)
    nc.finalize()
    return nc


# ==========================================================================
# Host orchestration
# ==========================================================================

_CACHE = {}
LAST_EXEC_NS = None
LAST_EXEC_NS1 = None
LAST_EXEC_NS2 = None


def _prep(inp):
    f = np.float32
    bf16 = ml_dtypes.bfloat16
    p = {}
    ctx_w = np.asarray(inp['ctx_w'], f)
    cn_w = np.asarray(inp['cn_w'], f)
    cn_b = np.asarray(inp['cn_b'], f)
    ctx_b = np.asarray(inp['ctx_b'], f)
    en_w = np.asarray(inp['en_w'], f)
    en_b = np.asarray(inp['en_b'], f)
    Wc = ctx_w[:, :C]
    We = ctx_w[:, C:]
    Wc2 = Wc * cn_w[None, :]
    kvec = cn_b @ Wc.T
    Wc3 = Wc2 - Wc2.mean(axis=1, keepdims=True)
    We2 = We * en_w[None, :]
    We3 = We2 - We2.mean(axis=1, keepdims=True)
    c0 = en_b @ We.T + ctx_b + kvec
    p['wc3'] = np.ascontiguousarray(Wc3.T.reshape(2, P, C))
    p['we3'] = np.ascontiguousarray(We3.T.reshape(4, P, C))
    p['c0v'] = np.ascontiguousarray(c0.reshape(C, 1))
    p['cnw'] = np.ascontiguousarray(cn_w.reshape(C, 1))
    p['cnb'] = np.ascontiguousarray(cn_b.reshape(C, 1))
    prev = np.asarray(inp['prev_context'], f)
    u0 = (prev - cn_b[None, :]) / cn_w[None, :]
    m0 = u0.mean(axis=1)
    u0c = u0 - m0[:, None]
    q = Wc2.sum(axis=1)
    p['ginit0'] = np.ascontiguousarray(u0c.T)
    p['hfix0'] = np.ascontiguousarray((m0[:, None] * q[None, :]).T)

    wqk = np.zeros((L, 2, P, 2 * C), bf16)
    bqk_h = np.zeros((L, 2 * C, 1), f)
    wv_h = np.zeros((L, 2, P, C), bf16)
    wao_h = np.zeros((L, 2, P, C), bf16)
    wf1_h = np.zeros((L, 2, P, I), bf16)
    bf1_h = np.zeros((L, I, 1), f)
    wf2_h = np.zeros((L, 8, P, C), bf16)
    bvo_h = np.zeros((L, C, 1), f)
    blay_h = np.zeros((L, 1, C), bf16)
    for l in range(L):
        qw = np.asarray(inp['qkv_w'][l], f)
        qb = np.asarray(inp['qkv_b'][l], f)
        l1w = np.asarray(inp['ln1_w'][l], f)
        l1b = np.asarray(inp['ln1_b'][l], f)
        A = qw * l1w[None, :]
        A = A - A.mean(axis=1, keepdims=True)
        bt = l1b @ qw.T + qb
        wqk[l] = A[0:2 * C].T.reshape(2, P, 2 * C).astype(bf16)
        bqk_h[l] = bt[0:2 * C].reshape(-1, 1)
        wv_h[l] = A[2 * C:3 * C].T.reshape(2, P, C).astype(bf16)
        bvo_h[l] = bt[2 * C:3 * C].reshape(-1, 1)
        wao_h[l] = np.asarray(inp['ao_w'][l], f).T.reshape(2, P, C).astype(bf16)
        l2w = np.asarray(inp['ln2_w'][l], f)
        l2b = np.asarray(inp['ln2_b'][l], f)
        f1w = np.asarray(inp['fc1_w'][l], f)
        A1 = f1w * l2w[None, :]
        A1 = A1 - A1.mean(axis=1, keepdims=True)
        wf1_h[l] = A1.T.reshape(2, P, I).astype(bf16)
        bf1_h[l] = (l2b @ f1w.T + np.asarray(inp['fc1_b'][l], f)).reshape(-1, 1)
        wf2_h[l] = np.asarray(inp['fc2_w'][l], f).T.reshape(8, P, C).astype(bf16)
        blay_h[l, 0] = (np.asarray(inp['ao_b'][l], f) +
                        np.asarray(inp['fc2_b'][l], f)).astype(bf16)
    p.update(wqk=wqk, bqk=bqk_h, wv=wv_h, wao=wao_h, wf1=wf1_h, bf1=bf1_h,
             wf2=wf2_h, bvo=bvo_h, blay=blay_h)

    pos = np.arange(S, dtype=f)
    inv = (1.0 / (10000.0 ** (np.arange(0, ROT, 2, dtype=f) / ROT))).astype(f)
    ang = pos[:, None] * inv[None, :]
    cosv = np.cos(ang).astype(f)
    sinv = np.sin(ang).astype(f)
    cosm = np.ones((P, S), f)
    sinm = np.zeros((P, S), f)
    for pp_ in range(P):
        d = pp_ % D
        if d < ROT:
            cosm[pp_] = cosv[:, d % (ROT // 2)]
            sinm[pp_] = sinv[:, d % (ROT // 2)]
    p['cosm'] = cosm
    p['sinm'] = sinm
    R32 = np.zeros((D, D), f)
    half = ROT // 2
    for d in range(half):
        R32[d, d + half] = -1.0
        R32[d + half, d] = 1.0
    Rfull = np.zeros((P, P), f)
    for jj in range(4):
        Rfull[jj * D:(jj + 1) * D, jj * D:(jj + 1) * D] = R32
    p['rsw'] = np.ascontiguousarray(Rfull.T).astype(bf16)
    tri = (np.arange(P)[:, None] <= np.arange(P)[None, :]).astype(f)
    p['tri01'] = tri.astype(bf16)
    p['flw'] = np.asarray(inp['fln_w'], f).reshape(C, 1)
    p['flb'] = np.asarray(inp['fln_b'], f).reshape(C, 1)
    ow = np.asarray(inp['out_w'], f).T
    p['outwt'] = []
    for h in range(2):
        owp = np.zeros((C, VHP), f)
        owp[:, :VH] = ow[:, h * VH:(h + 1) * VH]
        p['outwt'].append(np.ascontiguousarray(owp.reshape(2, P, VHP)).astype(bf16))
    return p


def kernel(**inputs):
    p = _prep(inputs)
    ids_np = np.asarray(inputs['input_ids']).astype(np.int64)

    if 'rec' not in _CACHE:
        _CACHE['rec'] = build_rec_program()
    nc1 = _CACHE['rec']
    embw_full = np.asarray(inputs['embed_w'], np.float32)
    zC_B = np.zeros((C, B), np.float32)
    in_maps = []
    for c in range(NCORE):
        s_k = c * CHUNK
        idsv = np.zeros((TBP,), np.int64)
        tok = ids_np[:, s_k:s_k + T]            # [B, T]
        idsv[:TB] = tok.T.reshape(-1)           # row t*B+b
        embw = np.ascontiguousarray(embw_full[idsv])   # staged rows [TBP, E]
        in_maps.append(dict(
            embw=embw, we3=p['we3'], wc3=p['wc3'], c0v=p['c0v'],
            ginit=(p['ginit0'] if c == 0 else zC_B),
            hfix=(p['hfix0'] if c == 0 else zC_B),
            cnw=p['cnw'], cnb=p['cnb']))
    import time as _time
    _t = _time.monotonic()
    res1 = run_bass_kernel_spmd(nc1, in_maps, core_ids=list(range(NCORE)))
    global LAST_EXEC_NS, LAST_EXEC_NS1
    LAST_EXEC_NS1 = res1.exec_time_ns or int((_time.monotonic() - _t) * 1e9)

    x = np.zeros((B, S, C), np.float32)
    for c in range(NCORE):
        s_k = c * CHUNK
        xo = np.asarray(res1.results[c]['xout'])          # [2, P, TB]
        xo = xo.reshape(C, T, B)
        t0 = 0 if c == 0 else W_WARM
        x[:, s_k + t0:s_k + T, :] = xo[:, t0:T, :].transpose(2, 1, 0)

    if 'lay' not in _CACHE:
        _CACHE['lay'] = build_layers_program()
    nc2 = _CACHE['lay']
    shared = dict(wqk=p['wqk'], bqk=p['bqk'], wv=p['wv'], wao=p['wao'],
                  wf1=p['wf1'], bf1=p['bf1'], wf2=p['wf2'], bvo=p['bvo'],
                  blay=p['blay'], cosm=p['cosm'], sinm=p['sinm'],
                  rsw=p['rsw'], tri01=p['tri01'], flw=p['flw'], flb=p['flb'])
    in_maps2 = []
    for c in range(NCORE):
        im = dict(shared)
        im['xb'] = np.ascontiguousarray(x[c // 2])
        im['outwt'] = p['outwt'][c % 2]
        in_maps2.append(im)
    _t = _time.monotonic()
    res2 = run_bass_kernel_spmd(nc2, in_maps2, core_ids=list(range(NCORE)))
    global LAST_EXEC_NS2
    LAST_EXEC_NS2 = res2.exec_time_ns or int((_time.monotonic() - _t) * 1e9)
    LAST_EXEC_NS = LAST_EXEC_NS1 + LAST_EXEC_NS2

    logits = np.zeros((B, S, V), np.float32)
    for c in range(NCORE):
        b, h = c // 2, c % 2
        lgv = np.asarray(res2.results[c]['lg'])
        logits[b, :, h * VH:(h + 1) * VH] = lgv[:, :VH]
    return logits

